# revision 34
# baseline (speedup 1.0000x reference)
"""Trainium2 Bass kernel for the Mamba2-style final-state chunk scan.

Math: the reference collapses to, per (b, h):
    out[p, n] = sum_t exp(sum_{t' > t} A[t']) * X[t, p] * B[t, n]
i.e. a weighted matmul over t (T=4096), with weights exp(strict suffix-sum
of A).  C is unused (the reference DCEs Y_diag).

Sharding: 128 (b, h) pairs -> 8 cores x 16 pairs.  All compute per pair is
independent; no collectives.  The host re-lays X/B/A into per-core "SBUF
image" layouts so every device DMA is fully contiguous.

Device plan per pair g (T split into 32 chunks of 128):
  Phase 0 (all pairs up front): suffix-sum weights
    - a_rows (32, 128) [chunk j on partitions, k in chunk on free]
    - PE transpose -> a_cols (128, 32)
    - suffix-sum argument via two matmuls accumulating in PSUM:
        M1: ss[k, j]  = sum_{k' > k} a[128 j + k']   (strict-lower ones)
        M2: ss[k, j] += sum_{j' > j} T[j']           (chunk totals)
    - w = exp(ss) on ACT
  Phase 1 (pipelined over pairs): X/B streamed in half-pair DMAs
    (X on the ACT HWDGE ring, B on the SP ring, stores via gpsimd SWDGE
    so no ring suffers store head-of-line blocking), X scaled in place
    by w (per-chunk per-partition broadcast on DVE), 32 accumulating
    matmuls per pair with B stationary:
      out[n, p] = sum_t B[t, n] * Xw[t, p]
    (moving free dim = 64 keeps the fp32 PE stream short; the host
    untransposes the tiny output at gather).

Cost-model timeline (TimelineSim): 148.8 us/core — DMA busy 142.0 us
(51.1 MB/core at the model's flat ~360 GB/s, gapless from 2.0 us to
143.8 us; stores batched 2 pairs/DMA for 512B runs), PE 58 us, DVE
44 us, ACT 12 us.  The remaining ~6.8 us is fixed latency: 2.0 us DMA
first-byte (hwdge + dge delay, exposed once), ~0.9 us completion
receipt on the last load, 4 matmuls + copy + store issue, and the
kernel drain.  Empirical steady-state on the axon TRN2 cores
(amplified wall-clock deltas): ~93-110 us/iteration.
"""

import os

import numpy as np

import concourse.mybir as mybir
from concourse import bacc
from concourse.bass_utils import run_bass_kernel_spmd
from concourse.masks import make_identity, make_lower_triangular
from concourse.tile import TileContext

N_CORES = 8
BATCH, T, H, P, N = 2, 4096, 64, 64, 128
CH = 128            # timesteps per device chunk (matmul contraction)
NCH = T // CH       # 32 chunks
PAIRS = BATCH * H   # 128
G = PAIRS // N_CORES  # 16 pairs per core
HALF = NCH // 2     # chunks per DMA/compute half

_nc_cache = None


def _build(reps=1):
    f32 = mybir.dt.float32
    nc = bacc.Bacc()
    X_d = nc.declare_dram_parameter("Xc", [G, CH, NCH, P], f32, isOutput=False)
    B_d = nc.declare_dram_parameter("Bc", [G, CH, NCH, N], f32, isOutput=False)
    A_d = nc.declare_dram_parameter("Ac", [G, NCH, CH], f32, isOutput=False)
    O_d = nc.declare_dram_parameter("Oc", [N, G, P], f32, isOutput=True)

    with TileContext(nc) as tc:
        with (
            tc.tile_pool(name="consts", bufs=1) as cpool,
            tc.tile_pool(name="abuf", bufs=1) as apool,
            tc.tile_pool(name="wbuf", bufs=1) as wbuf,
            tc.tile_pool(name="xb", bufs=4) as xpool,
            tc.tile_pool(name="bb", bufs=4) as bpool,
            tc.tile_pool(name="wsmall", bufs=4) as wpool,
            tc.tile_pool(name="osb", bufs=3) as opool,
            tc.tile_pool(name="ps_tr", bufs=2, space="PSUM") as ps_tr,
            tc.tile_pool(name="ps_w", bufs=2, space="PSUM") as ps_w,
            tc.tile_pool(name="ps_o", bufs=2, space="PSUM") as ps_o,
        ):
            # ---- constants ----
            sl128 = cpool.tile([CH, CH], f32)       # [k, i] = 1 iff k > i
            make_lower_triangular(nc, sl128, 1.0, diag=False)
            sl32 = cpool.tile([NCH, NCH], f32)      # [j', j] = 1 iff j' > j
            make_lower_triangular(nc, sl32, 1.0, diag=False)
            ident32 = cpool.tile([NCH, NCH], f32)
            make_identity(nc, ident32)
            ones32 = cpool.tile([NCH, CH], f32)
            nc.vector.memset(ones32, 1.0)

            # ---- phase 0: weights for all pairs ----
            # prefetch pair 0's first halves ahead of A so the bulk stream
            # owns the DMA engines from t=0 (the W phase has ~all of pair 0's
            # load time as slack)
            X0_sb = xpool.tile([CH, NCH, P], f32, tag="X_sb", name="X0_sb")
            B0_sb = bpool.tile([CH, NCH, N], f32, tag="B_sb", name="B0_sb")
            nc.scalar.dma_start(X0_sb[:, :HALF, :], X_d[0, :, :HALF, :])
            nc.sync.dma_start(B0_sb[:, :HALF, :], B_d[0, :, :HALF, :])

            A_sb = apool.tile([NCH, G, CH], f32)    # [j, g, k]
            nc.scalar.dma_start(A_sb, A_d.rearrange("g j k -> j g k"))

            w_all = wbuf.tile([CH, G, NCH], f32)    # per-pair weight cols
            for g in range(G):
                a_rows = A_sb[:, g, :]                       # (32, 128)
                ps_t = ps_tr.tile([CH, NCH], f32)
                nc.tensor.transpose(ps_t, a_rows, ident32)   # -> (128, 32)
                a_cols = wpool.tile([CH, NCH], f32, tag="a_cols")
                nc.scalar.copy(a_cols, ps_t)

                Tg = wpool.tile([NCH, 1], f32, tag="Tg")     # chunk totals
                nc.vector.reduce_sum(Tg, a_rows, axis=mybir.AxisListType.X)
                Tb = wpool.tile([NCH, CH], f32, tag="Tb")    # totals bcast
                nc.vector.tensor_scalar_mul(Tb, ones32, Tg[:, 0:1])

                ps_wt = ps_w.tile([CH, NCH], f32)
                nc.tensor.matmul(ps_wt, sl128, a_cols, start=True, stop=False)
                nc.tensor.matmul(ps_wt, Tb, sl32, start=False, stop=True,
                                 skip_group_check=True)
                nc.scalar.activation(w_all[:, g, :], ps_wt,
                                     mybir.ActivationFunctionType.Exp)

            # ---- phase 1: streamed weighted matmuls ----
            # matmul computes lhsT.T @ rhs with B as the stationary operand:
            # out[n, p] = sum_t B[t, n] * Xw[t, p]  (output transposed; the
            # host untransposes at gather).  Moving free dim = P (64) keeps
            # the PE stream short.
            o_sb = None
            for gi, g in enumerate([g for _ in range(reps) for g in range(G)]):
                if gi == 0:
                    X_sb, B_sb = X0_sb, B0_sb
                else:
                    X_sb = xpool.tile([CH, NCH, P], f32, tag="X_sb",
                                      name="X_sb")
                    B_sb = bpool.tile([CH, NCH, N], f32, tag="B_sb",
                                      name="B_sb")
                ps_out = ps_o.tile([N, P], f32)
                last = g == G - 1
                nsplit = 8 if last else 2
                spc = NCH // nsplit
                if last:
                    # final pair: all X pieces (and scales) ahead of the B
                    # pieces, so the post-last-load chain is matmuls only
                    for s in range(nsplit):
                        cs = slice(s * spc, (s + 1) * spc)
                        nc.scalar.dma_start(X_sb[:, cs, :], X_d[g, :, cs, :])
                        nc.vector.tensor_tensor(
                            X_sb[:, cs, :], X_sb[:, cs, :],
                            w_all[:, g, cs, None].to_broadcast((CH, spc, P)),
                            mybir.AluOpType.mult,
                        )
                    for s in range(nsplit):
                        cs = slice(s * spc, (s + 1) * spc)
                        nc.sync.dma_start(B_sb[:, cs, :], B_d[g, :, cs, :])
                        for c in range(s * spc, (s + 1) * spc):
                            nc.tensor.matmul(ps_out, B_sb[:, c, :],
                                             X_sb[:, c, :],
                                             start=(c == 0),
                                             stop=(c == NCH - 1))
                else:
                    for s in range(nsplit):
                        cs = slice(s * spc, (s + 1) * spc)
                        if gi == 0 and s == 0:
                            pass  # already prefetched above
                        else:
                            nc.scalar.dma_start(X_sb[:, cs, :],
                                                X_d[g, :, cs, :])
                            nc.sync.dma_start(B_sb[:, cs, :],
                                              B_d[g, :, cs, :])
                        # in-place scale: X *= w (broadcast over p), quarter
                        # granularity so matmuls start early
                        QU = max(spc // 2, 1)
                        for q0 in range(s * spc, (s + 1) * spc, QU):
                            qs = slice(q0, q0 + QU)
                            nc.vector.tensor_tensor(
                                X_sb[:, qs, :], X_sb[:, qs, :],
                                w_all[:, g, qs, None].to_broadcast(
                                    (CH, QU, P)),
                                mybir.AluOpType.mult,
                            )
                            for c in range(q0, q0 + QU):
                                nc.tensor.matmul(ps_out, B_sb[:, c, :],
                                                 X_sb[:, c, :],
                                                 start=(c == 0),
                                                 stop=(c == NCH - 1))
                # stores ride gpsimd SWDGE (off both HWDGE load rings),
                # batched two pairs per DMA for 512B runs; the final store
                # takes the idle SP ring's lower first-byte
                if g % 2 == 0:
                    o_sb = opool.tile([N, 2, P], f32, name="o_sb")
                nc.scalar.copy(o_sb[:, g % 2, :], ps_out)
                if g % 2 == 1:
                    store_eng = nc.sync if g == G - 1 else nc.gpsimd
                    store_eng.dma_start(O_d[:, g - 1:g + 1, :], o_sb)
    nc.finalize()
    return nc


def _get_nc():
    global _nc_cache
    if _nc_cache is None:
        _nc_cache = _build()
    return _nc_cache


def _shard(X, A, B):
    # host relayout to per-pair SBUF-image layouts (contiguous device DMAs)
    #   X: (b, (c k), h, p) -> (pair, k, c, p)
    Xr = X.reshape(BATCH, NCH, CH, H, P).transpose(0, 3, 2, 1, 4) \
          .reshape(PAIRS, CH, NCH, P)
    Br = B.reshape(BATCH, NCH, CH, H, N).transpose(0, 3, 2, 1, 4) \
          .reshape(PAIRS, CH, NCH, N)
    Ar = A.reshape(BATCH, NCH, CH, H).transpose(0, 3, 1, 2) \
          .reshape(PAIRS, NCH, CH)
    in_maps = []
    for i in range(N_CORES):
        sl = slice(i * G, (i + 1) * G)
        in_maps.append({
            "Xc": np.ascontiguousarray(Xr[sl]),
            "Bc": np.ascontiguousarray(Br[sl]),
            "Ac": np.ascontiguousarray(Ar[sl]),
        })
    return in_maps


def kernel(X, A, B, C=None, **_unused):
    # NTFF trace hooks are unavailable in this container; make sure a stray
    # BASS_TRACE env cannot route run_bass_kernel_spmd into that path.
    os.environ["BASS_NEVER_TRACE"] = "1"
    X = np.asarray(X, dtype=np.float32)
    A = np.asarray(A, dtype=np.float32)
    B = np.asarray(B, dtype=np.float32)

    in_maps = _shard(X, A, B)
    nc = _get_nc()
    res = run_bass_kernel_spmd(nc, in_maps, list(range(N_CORES)))
    # per-core (N, G, P) -> (pair, P, N)
    O = np.concatenate([r["Oc"] for r in res.results], axis=1)  # (N, 128, P)
    return np.ascontiguousarray(
        O.transpose(1, 2, 0).reshape(BATCH, H, P, N))


# revision 37
# speedup vs baseline: 4.1166x; 4.1166x over previous
"""Trainium2 Bass kernel for the Mamba2-style final-state chunk scan.

Math: the reference collapses to, per (b, h):
    out[p, n] = sum_t exp(sum_{t' > t} A[t']) * X[t, p] * B[t, n]
i.e. a weighted matmul over t (T=4096), with weights exp(strict suffix-sum
of A).  C is unused (the reference DCEs Y_diag).

Truncation (the big lever): A <= 0, so the weights decay exponentially
going back in time.  The host computes the exact per-pair suffix-sums of
A in float64 and keeps only the trailing chunks whose weights can exceed
e^-THR (THR=30): every dropped term is < e^-30 ~ 1e-13, and the summed
dropped weight is ~1e-12 — far below f32 resolution of the O(10) outputs
(the reference's own f32 arithmetic rounds these identically to zero
influence).  For the problem's distribution (|A| mean ~0.08) this keeps
K ~ 4 of 32 chunks, an ~8x DMA reduction; K is computed from the actual
input at run time, so atypical inputs simply get a larger K (up to the
full 32 = untruncated kernel) and stay exactly correct.

Sharding: 128 (b, h) pairs -> 8 cores x 16 pairs, no communication.  The
host re-lays the kept chunks of X/B/A into per-core "SBUF image" layouts
so every device DMA is fully contiguous.

Device plan per pair g (kept window of K chunks of 128 timesteps):
  Phase 0: weights w = exp(strict suffix-sum) for all pairs via a PE
    transpose of the A rows, two PSUM-accumulating matmuls against
    strict-lower-triangular ones masks (within-chunk suffix + later-chunk
    totals; the suffix never references dropped chunks since they are
    earlier in time), and exp on ACT.
  Phase 1: X/B streamed per pair (X on the ACT HWDGE ring, B on the SP
    ring, stores via gpsimd SWDGE batched two pairs per DMA for 512B
    runs), X scaled in place by w (per-chunk per-partition broadcast on
    DVE), K accumulating matmuls per pair with B stationary:
      out[n, p] = sum_t B[t, n] * Xw[t, p]
    (moving free dim = 64 keeps the fp32 PE stream short; the host
    untransposes the tiny output at gather).
"""

import os

import numpy as np

import concourse.mybir as mybir
from concourse import bacc
from concourse.bass_utils import run_bass_kernel_spmd
from concourse.masks import make_identity, make_lower_triangular
from concourse.tile import TileContext

N_CORES = 8
BATCH, T, H, P, N = 2, 4096, 64, 64, 128
CH = 128            # timesteps per device chunk (matmul contraction)
NCH = T // CH       # 32 chunks in the full sequence
PAIRS = BATCH * H   # 128
G = PAIRS // N_CORES  # 16 pairs per core
THR = 34.0          # keep timesteps with weight > e^-THR

_nc_cache = {}


def _build(kc, reps=1):
    """Build the kernel for a kept window of `kc` chunks per pair."""
    f32 = mybir.dt.float32
    nc = bacc.Bacc()
    X_d = nc.declare_dram_parameter("Xc", [G, CH, kc, P], f32, isOutput=False)
    B_d = nc.declare_dram_parameter("Bc", [G, CH, kc, N], f32, isOutput=False)
    A_d = nc.declare_dram_parameter("Ac", [G, kc, CH], f32, isOutput=False)
    O_d = nc.declare_dram_parameter("Oc", [N, G, P], f32, isOutput=True)

    with TileContext(nc) as tc:
        with (
            tc.tile_pool(name="consts", bufs=1) as cpool,
            tc.tile_pool(name="abuf", bufs=1) as apool,
            tc.tile_pool(name="wbuf", bufs=1) as wbuf,
            tc.tile_pool(name="xb", bufs=4) as xpool,
            tc.tile_pool(name="bb", bufs=4) as bpool,
            tc.tile_pool(name="wsmall", bufs=4) as wpool,
            tc.tile_pool(name="osb", bufs=3) as opool,
            tc.tile_pool(name="ps_tr", bufs=2, space="PSUM") as ps_tr,
            tc.tile_pool(name="ps_w", bufs=2, space="PSUM") as ps_w,
            tc.tile_pool(name="ps_o", bufs=2, space="PSUM") as ps_o,
        ):
            # ---- constants ----
            sl128 = cpool.tile([CH, CH], f32)       # [k, i] = 1 iff k > i
            make_lower_triangular(nc, sl128, 1.0, diag=False)
            slk = cpool.tile([kc, kc], f32)         # [j', j] = 1 iff j' > j
            make_lower_triangular(nc, slk, 1.0, diag=False)
            identk = cpool.tile([kc, kc], f32)
            make_identity(nc, identk)
            onesk = cpool.tile([kc, CH], f32)
            nc.vector.memset(onesk, 1.0)

            # ---- phase 0: weights for all pairs ----
            # prefetch pair 0's loads ahead of A so the bulk stream owns
            # the DMA engines from t=0
            X0_sb = xpool.tile([CH, kc, P], f32, tag="X_sb", name="X0_sb")
            B0_sb = bpool.tile([CH, kc, N], f32, tag="B_sb", name="B0_sb")
            nc.scalar.dma_start(X0_sb, X_d[0])
            nc.sync.dma_start(B0_sb, B_d[0])

            A_sb = apool.tile([kc, G, CH], f32)     # [j, g, k]
            nc.scalar.dma_start(A_sb, A_d.rearrange("g j k -> j g k"))

            w_all = wbuf.tile([CH, G, kc], f32)     # per-pair weight cols
            for g in range(G):
                a_rows = A_sb[:, g, :]                       # (kc, 128)
                ps_t = ps_tr.tile([CH, kc], f32)
                nc.tensor.transpose(ps_t, a_rows, identk)    # -> (128, kc)
                a_cols = wpool.tile([CH, kc], f32, tag="a_cols")
                nc.scalar.copy(a_cols, ps_t)

                Tg = wpool.tile([kc, 1], f32, tag="Tg")      # chunk totals
                nc.vector.reduce_sum(Tg, a_rows, axis=mybir.AxisListType.X)
                Tb = wpool.tile([kc, CH], f32, tag="Tb")     # totals bcast
                nc.vector.tensor_scalar_mul(Tb, onesk, Tg[:, 0:1])

                ps_wt = ps_w.tile([CH, kc], f32)
                nc.tensor.matmul(ps_wt, sl128, a_cols, start=True, stop=False)
                nc.tensor.matmul(ps_wt, Tb, slk, start=False, stop=True,
                                 skip_group_check=True)
                nc.scalar.activation(w_all[:, g, :], ps_wt,
                                     mybir.ActivationFunctionType.Exp)

            # ---- phase 1: streamed weighted matmuls ----
            o_sb = None
            for gi, g in enumerate([g for _ in range(reps) for g in range(G)]):
                if gi == 0:
                    X_sb, B_sb = X0_sb, B0_sb
                else:
                    X_sb = xpool.tile([CH, kc, P], f32, tag="X_sb",
                                      name="X_sb")
                    B_sb = bpool.tile([CH, kc, N], f32, tag="B_sb",
                                      name="B_sb")
                    nc.scalar.dma_start(X_sb, X_d[g])
                    nc.sync.dma_start(B_sb, B_d[g])
                # in-place scale: X *= w (broadcast over p)
                nc.vector.tensor_tensor(
                    X_sb, X_sb,
                    w_all[:, g, :, None].to_broadcast((CH, kc, P)),
                    mybir.AluOpType.mult,
                )
                ps_out = ps_o.tile([N, P], f32)
                for c in range(kc):
                    nc.tensor.matmul(ps_out, B_sb[:, c, :], X_sb[:, c, :],
                                     start=(c == 0), stop=(c == kc - 1))
                # stores ride gpsimd SWDGE (off both HWDGE load rings),
                # batched two pairs per DMA for 512B runs; the final store
                # takes the idle SP ring's lower first-byte
                if g % 2 == 0:
                    o_sb = opool.tile([N, 2, P], f32, name="o_sb")
                nc.scalar.copy(o_sb[:, g % 2, :], ps_out)
                if g % 2 == 1:
                    store_eng = nc.sync if g == G - 1 else nc.gpsimd
                    store_eng.dma_start(O_d[:, g - 1:g + 1, :], o_sb)
    nc.finalize()
    return nc


def _get_nc(kc):
    if kc not in _nc_cache:
        _nc_cache[kc] = _build(kc)
    return _nc_cache[kc]


def _window_chunks(A):
    """Smallest K such that every timestep with weight > e^-THR lies in
    the last K chunks (exact, from the data; float64)."""
    S = np.cumsum(A[:, ::-1, :].astype(np.float64), axis=1)[:, ::-1, :]
    suf = S - A                      # strict suffix-sum after t
    keep = suf > -THR                # monotone in t (A <= 0)
    tmin = np.argmax(keep, axis=1)   # first kept t per (b, h); last t
    cmin = int(tmin.min()) // CH     # always kept (empty suffix = 0)
    return min(NCH, max(1, NCH - cmin) + 1)  # +1 chunk safety margin


def _shard(X, A, B, kc):
    # keep only the trailing kc chunks, re-laid to per-pair SBUF-image
    # layouts (contiguous device DMAs):  X: (b, (c k), h, p) -> (pair, k, c, p)
    c0 = NCH - kc
    Xr = X.reshape(BATCH, NCH, CH, H, P)[:, c0:].transpose(0, 3, 2, 1, 4) \
          .reshape(PAIRS, CH, kc, P)
    Br = B.reshape(BATCH, NCH, CH, H, N)[:, c0:].transpose(0, 3, 2, 1, 4) \
          .reshape(PAIRS, CH, kc, N)
    Ar = A.reshape(BATCH, NCH, CH, H)[:, c0:].transpose(0, 3, 1, 2) \
          .reshape(PAIRS, kc, CH)
    in_maps = []
    for i in range(N_CORES):
        sl = slice(i * G, (i + 1) * G)
        in_maps.append({
            "Xc": np.ascontiguousarray(Xr[sl]),
            "Bc": np.ascontiguousarray(Br[sl]),
            "Ac": np.ascontiguousarray(Ar[sl]),
        })
    return in_maps


def kernel(X, A, B, C=None, **_unused):
    # NTFF trace hooks are unavailable in this container; make sure a stray
    # BASS_TRACE env cannot route run_bass_kernel_spmd into that path.
    os.environ["BASS_NEVER_TRACE"] = "1"
    X = np.asarray(X, dtype=np.float32)
    A = np.asarray(A, dtype=np.float32)
    B = np.asarray(B, dtype=np.float32)

    kc = _window_chunks(A)
    in_maps = _shard(X, A, B, kc)
    nc = _get_nc(kc)
    res = run_bass_kernel_spmd(nc, in_maps, list(range(N_CORES)))
    # per-core (N, G, P) -> (pair, P, N)
    O = np.concatenate([r["Oc"] for r in res.results], axis=1)  # (N, 128, P)
    return np.ascontiguousarray(
        O.transpose(1, 2, 0).reshape(BATCH, H, P, N))
